# revision 3
# baseline (speedup 1.0000x reference)
"""Trainium2 Bass kernel for nn_AttentionBlock (sparse_attention).

Reference computation per batch b (channels-first x[b]: [C=512, T=4096]):
    xt = x[b].T                                  # [T, C]
    q = xt @ Wq.T + bq ; k = xt @ Wk.T + bk      # [T, 512]
    v = xt @ Wv.T + bv                           # [T, 512]
    S = q @ k.T / sqrt(512), causal (j <= i)     # [T, T]
    P = softmax(S, axis=QUERY i)  (per-column normalization)
    act = P @ v                                  # [T, 512]
    out[b] = concat(x[b], act.T, axis=0)         # [1024, T]

Sharding: pure data-parallel over batch B=8 across the 8 NeuronCores
(one batch per core, no collectives).

Per-core algorithm (matmuls fp16 / optionally fp8-DoubleRow, f32 PSUM):
  1. QKV projections from x (host-cast to fp16), producing
     Q^T,K^T: [512, T] (head-dim on partitions) and V: [T, 512]
     (time on partitions).  1/sqrt(512) score scale folded into
     Wq,bq,Wk,bk on host (split as 512**-0.25 on each side).
  2. ST[j,i] = K^T.T @ Q^T strips (j-chunk of 128 rows at a time,
     i from the diagonal to T).  Column-softmax over i = free-axis
     ops: additive causal mask on the diagonal 128x128, then
     exp(s - 4) on ScalarE; row sums Z_j reduced on VectorE.
     P~ = exp(ST - 4) stored to DRAM scratch.
  3. V rows scaled by 1/Z_j (folds softmax denominator into V).
  4. act^T[v,i] = sum_j V'[j,v] * P~[j,i]: PSUM-accumulated matmuls
     over j-chunks, streaming P~ tiles back from DRAM.
  5. out rows 0..511 are a DRAM->DRAM copy of x[b]; rows 512..1023
     get act^T.
"""

import math

import numpy as np

import concourse.bass as bass
import concourse.mybir as mybir
from concourse import bacc, tile
from concourse.bass_utils import run_bass_kernel_spmd

P = 128
C = 512
T = 4096
KDIM = 512
VDIM = 512
NCC = C // P      # 4 contraction chunks over channels
NKK = KDIM // P   # 4 chunks of head dim
NTC = T // P      # 32 time chunks of 128
NIB = T // 512    # 8 i-blocks of 512
F16 = mybir.dt.float16
F32 = mybir.dt.float32
F8 = mybir.dt.float8e4
EXP_SHIFT = -4.0  # constant logit shift: softmax-invariant, keeps exp in range
MASK_NEG = -10000.0

# fp8e4 (TRN: max +-240) + DoubleRow for the two TxT phases.  Halves the
# TensorE streaming time of those phases; error budget validated in numpy
# (global rel err stays ~5e-3 vs the 2e-2 gate).
PHASE1_FP8 = False  # S = K^T.T @ Q^T matmuls
PHASE2_FP8 = False  # act = V'^T @ P~ matmuls (also stores P~ as fp8)

_CACHE = {}


def _ts(i, size):
    return slice(i * size, (i + 1) * size)


def build_nc(phase1_fp8=None, phase2_fp8=None):
    p1f8 = PHASE1_FP8 if phase1_fp8 is None else phase1_fp8
    p2f8 = PHASE2_FP8 if phase2_fp8 is None else phase2_fp8
    pt_dt = F8 if p2f8 else F16

    nc = bacc.Bacc(
        "TRN2",
        target_bir_lowering=False,
        debug=False,
        num_devices=8,
    )

    x16_d = nc.declare_dram_parameter("x16", [C, T], F16, isOutput=False)
    x32_d = nc.declare_dram_parameter("x32", [C, T], F32, isOutput=False)
    wqt_d = nc.declare_dram_parameter("wqt", [C, KDIM], F16, isOutput=False)
    wkt_d = nc.declare_dram_parameter("wkt", [C, KDIM], F16, isOutput=False)
    wvt_d = nc.declare_dram_parameter("wvt", [C, VDIM], F16, isOutput=False)
    bq_d = nc.declare_dram_parameter("bq", [P, NKK], F32, isOutput=False)
    bk_d = nc.declare_dram_parameter("bk", [P, NKK], F32, isOutput=False)
    bv_d = nc.declare_dram_parameter("bv", [P, VDIM], F32, isOutput=False)
    mask_d = nc.declare_dram_parameter("mask", [P, P], F32, isOutput=False)
    out_d = nc.declare_dram_parameter("out", [C + VDIM, T], F32, isOutput=True)

    with tile.TileContext(nc) as tc:
        from contextlib import ExitStack

        with ExitStack() as ctx:
            singles = ctx.enter_context(tc.tile_pool(name="singles", bufs=1))

            def single(shape, dtype, tag):
                return singles.tile(shape, dtype, name=tag, tag=tag)

            x16_s = [single([P, T], F16, f"x16s{c}") for c in range(NCC)]
            wq_s = [single([P, KDIM], F16, f"wqs{c}") for c in range(NCC)]
            wk_s = [single([P, KDIM], F16, f"wks{c}") for c in range(NCC)]
            wv_s = [single([P, VDIM], F16, f"wvs{c}") for c in range(NCC)]
            bq_s = single([P, NKK], F32, "bqs")
            bk_s = single([P, NKK], F32, "bks")
            bv_s = single([P, VDIM], F32, "bvs")
            mask_s = single([P, P], F32, "masks")
            if p1f8:
                # paired layout [p, u(2), i]: column u*T+i holds chunk kk=2*h+u
                qt8_s = [single([P, 2 * T], F8, f"qt8s{h}") for h in range(2)]
                kt8_s = [single([P, 2 * T], F8, f"kt8s{h}") for h in range(2)]
                qt_s = kt_s = None
            else:
                qt_s = [single([P, T], F16, f"qts{k}") for k in range(NKK)]
                kt_s = [single([P, T], F16, f"kts{k}") for k in range(NKK)]
            v_s = [single([P, VDIM], F16, f"vs{t}") for t in range(NTC)]
            if p2f8:
                # paired scaled-V [p, u(2), v]: column u*512+v holds chunk 2*h+u
                v8_s = [single([P, 2 * VDIM], F8, f"v8s{h}") for h in range(NTC // 2)]
            zr_s = single([P, NTC], F32, "zrs")
            zero_s = single([P, 384], pt_dt, "zeros")
            expshift_s = single([P, 1], F32, "expshift")
            nc.vector.memset(expshift_s, EXP_SHIFT)

            # ---- input DMAs (small weights first so PE can start ASAP;
            # the x->out copy-through is issued at the very end of the
            # program so it doesn't compete with input loads) ----
            nc.sync.dma_start(out=bq_s, in_=bq_d[:, :])
            nc.sync.dma_start(out=bk_s, in_=bk_d[:, :])
            nc.sync.dma_start(out=bv_s, in_=bv_d[:, :])
            nc.sync.dma_start(out=mask_s, in_=mask_d[:, :])
            for c in range(NCC):
                nc.sync.dma_start(out=wq_s[c], in_=wqt_d[_ts(c, P), :])
                nc.sync.dma_start(out=wk_s[c], in_=wkt_d[_ts(c, P), :])
                nc.sync.dma_start(out=wv_s[c], in_=wvt_d[_ts(c, P), :])
            # x16 split into 1024-col pieces, first pieces of every c-chunk
            # first, so the first QKV matmuls unblock early
            for piece in range(4):
                for c in range(NCC):
                    nc.sync.dma_start(
                        out=x16_s[c][:, _ts(piece, 1024)],
                        in_=x16_d[_ts(c, P), _ts(piece, 1024)],
                    )
            nc.vector.memset(zero_s, 0.0)

            # P~ strips in DRAM scratch, one tile per j-chunk so Tile's
            # dependency tracking stays per-strip.
            ptdram = ctx.enter_context(
                tc.tile_pool(name="ptdram", bufs=1, space="DRAM")
            )
            pt_d = [
                ptdram.tile([P, T], pt_dt, name=f"pt{j}", tag=f"pt{j}")
                for j in range(NTC)
            ]

            # ---- Phase QKV ----
            qkv_ps_cm = tc.tile_pool(name="qkv_ps", bufs=4, space="PSUM")
            qkv_ps = qkv_ps_cm.__enter__()
            for which, (w_s, b_s) in enumerate(((wq_s, bq_s), (wk_s, bk_s))):
                for kk in range(NKK):
                    for ib in range(NIB):
                        ps = qkv_ps.tile([P, 512], F32, tag="qkvps", name="ps_qk")
                        for c in range(NCC):
                            nc.tensor.matmul(
                                ps,
                                lhsT=w_s[c][:, _ts(kk, P)],
                                rhs=x16_s[c][:, _ts(ib, 512)],
                                start=(c == 0),
                                stop=(c == NCC - 1),
                            )
                        if p1f8:
                            dst8 = (qt8_s, kt8_s)[which]
                            dst_ap = dst8[kk // 2][
                                :, (kk % 2) * T + ib * 512 : (kk % 2) * T + ib * 512 + 512
                            ]
                        else:
                            dst_ap = ((qt_s, kt_s)[which])[kk][:, _ts(ib, 512)]
                        nc.scalar.activation(
                            dst_ap,
                            ps,
                            mybir.ActivationFunctionType.Identity,
                            bias=b_s[:, kk : kk + 1],
                            scale=1.0,
                        )
            for t in range(NTC):
                ps = qkv_ps.tile([P, 512], F32, tag="qkvps", name="ps_v")
                for c in range(NCC):
                    nc.tensor.matmul(
                        ps,
                        lhsT=x16_s[c][:, _ts(t, P)],
                        rhs=wv_s[c],
                        start=(c == 0),
                        stop=(c == NCC - 1),
                    )
                nc.vector.tensor_add(v_s[t], ps, bv_s)
            qkv_ps_cm.__exit__(None, None, None)

            # ---- Phase 1 + 2 interleaved: score strips feed act blocks ----
            # act block ib is emitted right after strip jc=4*ib+3, so PE's
            # act matmuls overlap ScalarE's exp of later strips and the
            # act-PSUM eviction happens while the next strips compute.
            s_ps = ctx.enter_context(tc.tile_pool(name="s_ps", bufs=4, space="PSUM"))
            ptb_pool = ctx.enter_context(tc.tile_pool(name="ptb", bufs=4))
            zp_pool = ctx.enter_context(tc.tile_pool(name="zp", bufs=4))
            act_ps = ctx.enter_context(
                tc.tile_pool(name="act_ps", bufs=1, space="PSUM")
            )
            pti_pool = ctx.enter_context(tc.tile_pool(name="pti", bufs=8))
            ob_pool = ctx.enter_context(tc.tile_pool(name="ob", bufs=4))

            def emit_act_block(ib):
                njc = 4 * (ib + 1)
                pss = [
                    act_ps.tile([P, 512], F32, tag=f"aps{v}", name=f"aps{v}")
                    for v in range(4)
                ]
                if p2f8:
                    for m in range(njc // 2):
                        pti = pti_pool.tile([P, 1024], F8, tag="pti", name="pti")
                        nc.sync.dma_start(
                            out=pti[:, 0:512], in_=pt_d[2 * m][:, _ts(ib, 512)]
                        )
                        nc.sync.dma_start(
                            out=pti[:, 512:1024],
                            in_=pt_d[2 * m + 1][:, _ts(ib, 512)],
                        )
                        rhs3 = pti.rearrange("p (u n) -> p u n", u=2)
                        for vc in range(4):
                            lhs3 = v8_s[m].rearrange("p (u n) -> p u n", u=2)[
                                :, :, _ts(vc, P)
                            ]
                            nc.tensor.matmul(
                                pss[vc],
                                lhsT=lhs3,
                                rhs=rhs3,
                                start=(m == 0),
                                stop=(m == njc // 2 - 1),
                                perf_mode=mybir.MatmulPerfMode.DoubleRow,
                            )
                else:
                    for jc in range(njc):
                        pti = pti_pool.tile([P, 512], F16, tag="pti", name="pti")
                        nc.sync.dma_start(out=pti, in_=pt_d[jc][:, _ts(ib, 512)])
                        for vc in range(4):
                            nc.tensor.matmul(
                                pss[vc],
                                lhsT=v_s[jc][:, _ts(vc, P)],
                                rhs=pti,
                                start=(jc == 0),
                                stop=(jc == njc - 1),
                            )
                for vc in range(4):
                    ob = ob_pool.tile([P, 512], F32, tag="ob", name="ob")
                    if vc % 2 == 0:
                        nc.scalar.copy(ob, pss[vc])
                    else:
                        nc.vector.tensor_copy(ob, pss[vc])
                    nc.sync.dma_start(
                        out=out_d[C + vc * P : C + (vc + 1) * P, _ts(ib, 512)],
                        in_=ob,
                    )

            for jc in range(NTC):
                i0 = P * jc
                a0 = 512 * (jc // 4)
                r = jc % 4
                if r > 0:
                    # zero the never-written corner so phase 2 reads are clean
                    nc.sync.dma_start(
                        out=pt_d[jc][:, a0:i0], in_=zero_s[:, 0 : P * r]
                    )
                starts = [i0] + list(range(a0 + 512, T, 512))
                nch = len(starts)
                zp = zp_pool.tile([P, NIB], F32, tag="zp", name="zp")
                for ci, a in enumerate(starts):
                    b = a0 + 512 * (ci + 1)
                    w = b - a
                    ps = s_ps.tile([P, 512], F32, tag="sps", name="ps_s")
                    if p1f8:
                        for h in range(2):
                            lhs3 = kt8_s[h].rearrange("p (u n) -> p u n", u=2)[
                                :, :, i0 : i0 + P
                            ]
                            rhs3 = qt8_s[h].rearrange("p (u n) -> p u n", u=2)[
                                :, :, a:b
                            ]
                            nc.tensor.matmul(
                                ps[:, 0:w],
                                lhsT=lhs3,
                                rhs=rhs3,
                                start=(h == 0),
                                stop=(h == 1),
                                perf_mode=mybir.MatmulPerfMode.DoubleRow,
                            )
                    else:
                        for kk in range(NKK):
                            nc.tensor.matmul(
                                ps[:, 0:w],
                                lhsT=kt_s[kk][:, _ts(jc, P)],
                                rhs=qt_s[kk][:, a:b],
                                start=(kk == 0),
                                stop=(kk == NKK - 1),
                            )
                    if ci == 0:
                        nc.vector.tensor_add(ps[:, 0:P], ps[:, 0:P], mask_s)
                    ptb = ptb_pool.tile([P, 512], pt_dt, tag="ptb", name="ptb")
                    nc.scalar.activation(
                        ptb[:, 0:w],
                        ps[:, 0:w],
                        mybir.ActivationFunctionType.Exp,
                        bias=expshift_s[:, 0:1],
                        scale=1.0,
                    )
                    # Z partial on DVE so ScalarE (exp) stays ahead of PE
                    nc.vector.reduce_sum(
                        zp[:, ci : ci + 1], ptb[:, 0:w], axis=mybir.AxisListType.X
                    )
                    nc.sync.dma_start(out=pt_d[jc][:, a:b], in_=ptb[:, 0:w])
                z = zp_pool.tile([P, 1], F32, tag="zf", name="z")
                nc.vector.reduce_sum(z, zp[:, 0:nch], axis=mybir.AxisListType.X)
                nc.vector.reciprocal(zr_s[:, jc : jc + 1], z)
                # fold 1/Z_j into V rows (partition j)
                if p2f8:
                    nc.vector.tensor_scalar_mul(
                        v8_s[jc // 2][:, _ts(jc % 2, VDIM)],
                        v_s[jc],
                        zr_s[:, jc : jc + 1],
                    )
                else:
                    nc.vector.tensor_scalar_mul(
                        v_s[jc], v_s[jc], zr_s[:, jc : jc + 1]
                    )
                if jc % 4 == 3:
                    emit_act_block(jc // 4)

            # x copy-through rows 0..511 (DRAM->DRAM), issued last so it
            # never competes with latency-critical DMA traffic
            for c in range(NCC):
                for piece in range(2):
                    nc.sync.dma_start(
                        out=out_d[_ts(c, P), _ts(piece, 2048)],
                        in_=x32_d[_ts(c, P), _ts(piece, 2048)],
                    )

    nc.compile()
    return nc


def _host_inputs(x, Wq, bq, Wk, bk, Wv, bv):
    c4 = float(C) ** 0.25
    wqt = np.ascontiguousarray(Wq.T / c4).astype(np.float16)
    wkt = np.ascontiguousarray(Wk.T / c4).astype(np.float16)
    wvt = np.ascontiguousarray(Wv.T).astype(np.float16)
    bq_h = np.ascontiguousarray((bq / c4).reshape(NKK, P).T).astype(np.float32)
    bk_h = np.ascontiguousarray((bk / c4).reshape(NKK, P).T).astype(np.float32)
    bv_h = np.ascontiguousarray(np.tile(bv.astype(np.float32), (P, 1)))
    r = np.arange(P)
    mask = np.where(r[None, :] >= r[:, None], 0.0, MASK_NEG).astype(np.float32)
    in_maps = []
    for b in range(x.shape[0]):
        xb = np.ascontiguousarray(x[b]).astype(np.float32)
        in_maps.append(
            {
                "x16": xb.astype(np.float16),
                "x32": xb,
                "wqt": wqt,
                "wkt": wkt,
                "wvt": wvt,
                "bq": bq_h,
                "bk": bk_h,
                "bv": bv_h,
                "mask": mask,
            }
        )
    return in_maps


def kernel(x, Wq, bq, Wk, bk, Wv, bv, _trace=False, _tmpdir=None):
    import time as _time

    x = np.asarray(x, dtype=np.float32)
    if "nc" not in _CACHE:
        t0 = _time.time()
        _CACHE["nc"] = build_nc()
        print(f"[kernel] build_nc done in {_time.time() - t0:.1f}s", flush=True)
    nc = _CACHE["nc"]
    in_maps = _host_inputs(
        x,
        np.asarray(Wq, np.float32),
        np.asarray(bq, np.float32),
        np.asarray(Wk, np.float32),
        np.asarray(bk, np.float32),
        np.asarray(Wv, np.float32),
        np.asarray(bv, np.float32),
    )
    t0 = _time.time()
    res = run_bass_kernel_spmd(
        nc, in_maps, core_ids=list(range(8)), trace=_trace, tmpdir=_tmpdir
    )
    print(f"[kernel] run done in {_time.time() - t0:.1f}s", flush=True)
    _CACHE["last_result"] = res
    out = np.stack([r["out"] for r in res.results]).astype(np.float32)
    return out



# revision 7
# speedup vs baseline: 1.5534x; 1.5534x over previous
"""Trainium2 Bass kernel for nn_AttentionBlock (sparse_attention).

Reference computation per batch b (channels-first x[b]: [C=512, T=4096]):
    xt = x[b].T                                  # [T, C]
    q = xt @ Wq.T + bq ; k = xt @ Wk.T + bk      # [T, 512]
    v = xt @ Wv.T + bv                           # [T, 512]
    S = q @ k.T / sqrt(512), causal (j <= i)     # [T, T]
    P = softmax(S, axis=QUERY i)  (per-column normalization)
    act = P @ v                                  # [T, 512]
    out[b] = concat(x[b], act.T, axis=0)         # [1024, T]

Sharding: pure data-parallel over batch B=8 across the 8 NeuronCores
(one batch per core, no collectives).

v2 per-core algorithm — all matmuls fp8e4 DoubleRow (f32 PSUM):
  1. QKV projections from x8 (host-cast fp8).  Contraction over C=512
     as 2 DoubleRow pairs.  Q^T,K^T stored fp8 paired over head-dim
     chunks; V rows fp16 (time on partitions).  Q/K eviction on DVE:
     (psum * 1/c4) + b/c4 -> fp8 (c4 = 512**0.25 splits the score
     scale between q and k).
  2. Score strips ST[j,i] = K^T.T @ Q^T (j-chunk of 128 keys, i from
     the diagonal to T, 512-col chunks, h-outer groups of 4 chunks to
     amortize LDWEIGHTS).  Column softmax over i: additive causal mask
     on the diagonal block, exp(s - 4) on ScalarE with accum_out
     producing the Z_j partial sums for free.  P~ = exp(ST-4) stays in
     SBUF as fp8 pair tiles (triangle = 72KB/partition).
  3. V rows scaled by 1/Z_j on DVE, clipped to +-240 (fp8e4 has no
     saturation: v/Z overflows for late columns where Z is tiny), cast
     fp8 into paired v8 tiles.
  4. act^T[v,i] = sum_j V'[j,v] * P~[j,i]: PSUM-accumulated DoubleRow
     matmuls reading P~/V' straight from SBUF.  Act-block matmuls are
     interleaved between score-strip chunk groups so TensorE never
     stalls on the ScalarE exp chain.  Eviction on GpSimd, DMA to out.
  5. out rows 0..511 are a DRAM->DRAM copy of x[b], issued up front.
"""

import math

import numpy as np

import concourse.bass as bass
import concourse.mybir as mybir
from concourse import bacc, tile
from concourse.bass_utils import run_bass_kernel_spmd

P = 128
C = 512
T = 4096
KDIM = 512
VDIM = 512
NTC = T // P      # 32 time chunks of 128
NIB = T // 512    # 8 i-blocks of 512
F16 = mybir.dt.float16
F32 = mybir.dt.float32
F8 = mybir.dt.float8e4
EXP_SHIFT = -4.0  # constant logit shift: softmax-invariant, keeps exp in range
MASK_NEG = -10000.0
C4 = float(C) ** 0.25
FP8MAX = 240.0
DR = mybir.MatmulPerfMode.DoubleRow

_CACHE = {}


def _ts(i, size):
    return slice(i * size, (i + 1) * size)


def build_nc():
    nc = bacc.Bacc(
        "TRN2",
        target_bir_lowering=False,
        debug=False,
        num_devices=8,
    )

    x8_d = nc.declare_dram_parameter("x8", [C, T], F8, isOutput=False)
    x32_d = nc.declare_dram_parameter("x32", [C, T], F32, isOutput=False)
    wq8_d = nc.declare_dram_parameter("wq8", [C, KDIM], F8, isOutput=False)
    wk8_d = nc.declare_dram_parameter("wk8", [C, KDIM], F8, isOutput=False)
    wv8_d = nc.declare_dram_parameter("wv8", [C, VDIM], F8, isOutput=False)
    bq_d = nc.declare_dram_parameter("bq", [P, 4], F32, isOutput=False)
    bk_d = nc.declare_dram_parameter("bk", [P, 4], F32, isOutput=False)
    bv_d = nc.declare_dram_parameter("bv", [P, VDIM], F32, isOutput=False)
    mask_d = nc.declare_dram_parameter("mask", [P, P], F32, isOutput=False)
    out_d = nc.declare_dram_parameter("out", [C + VDIM, T], F32, isOutput=True)

    def re2(ap):
        return ap.rearrange("p (u n) -> p u n", u=2)

    with tile.TileContext(nc) as tc:
        from contextlib import ExitStack

        with ExitStack() as ctx:
            singles = ctx.enter_context(tc.tile_pool(name="singles", bufs=1))

            def single(shape, dtype, tag):
                return singles.tile(shape, dtype, name=tag, tag=tag)

            # paired fp8 layouts: plane u of tile h holds 128-chunk 2h+u
            x8_s = [single([P, 2 * T], F8, f"x8s{h}") for h in range(2)]
            wq8_s = [single([P, 2 * KDIM], F8, f"wq8s{h}") for h in range(2)]
            wk8_s = [single([P, 2 * KDIM], F8, f"wk8s{h}") for h in range(2)]
            wv8_s = [single([P, 2 * VDIM], F8, f"wv8s{h}") for h in range(2)]
            bq_s = single([P, 4], F32, "bqs")
            bk_s = single([P, 4], F32, "bks")
            bv_s = single([P, VDIM], F32, "bvs")
            mask_s = single([P, P], F32, "masks")
            qt8_s = [single([P, 2 * T], F8, f"qt8s{h}") for h in range(2)]
            kt8_s = [single([P, 2 * T], F8, f"kt8s{h}") for h in range(2)]
            v16_s = [single([P, VDIM], F16, f"v16s{t}") for t in range(NTC)]
            v8_s = [single([P, 2 * VDIM], F8, f"v8s{m}") for m in range(NTC // 2)]
            # P~ fp8 pair tiles: pair m holds strips jc=2m,2m+1; valid
            # i >= a0 = 512*(m//2); plane length Lm = T - a0
            pt8_s = []
            for m in range(NTC // 2):
                Lm = T - 512 * (m // 2)
                pt8_s.append(single([P, 2 * Lm], F8, f"pt8s{m}"))
            zr_s = single([P, NTC], F32, "zrs")
            expshift_s = single([P, 1], F32, "expshift")
            nc.vector.memset(expshift_s, EXP_SHIFT)

            # ---- input DMAs (weights/bias first so PE starts ASAP) ----
            nc.sync.dma_start(out=bq_s, in_=bq_d[:, :])
            nc.sync.dma_start(out=bk_s, in_=bk_d[:, :])
            nc.sync.dma_start(out=bv_s, in_=bv_d[:, :])
            nc.sync.dma_start(out=mask_s, in_=mask_d[:, :])
            for h in range(2):
                for u in range(2):
                    cc = 2 * h + u
                    nc.sync.dma_start(
                        out=wq8_s[h][:, _ts(u, KDIM)], in_=wq8_d[_ts(cc, P), :]
                    )
                    nc.sync.dma_start(
                        out=wk8_s[h][:, _ts(u, KDIM)], in_=wk8_d[_ts(cc, P), :]
                    )
                    nc.sync.dma_start(
                        out=wv8_s[h][:, _ts(u, VDIM)], in_=wv8_d[_ts(cc, P), :]
                    )
            # x8: plane (h, u) <- x rows of c-chunk 2h+u, in 2048-col pieces
            for piece in range(2):
                for h in range(2):
                    for u in range(2):
                        cc = 2 * h + u
                        nc.sync.dma_start(
                            out=x8_s[h][:, u * T + piece * 2048 : u * T + piece * 2048 + 2048],
                            in_=x8_d[_ts(cc, P), _ts(piece, 2048)],
                        )
            # x copy-through rows 0..511 (DRAM->DRAM), issued early:
            # transfers overlap the whole kernel
            for c in range(4):
                for piece in range(2):
                    nc.sync.dma_start(
                        out=out_d[_ts(c, P), _ts(piece, 2048)],
                        in_=x32_d[_ts(c, P), _ts(piece, 2048)],
                    )

            # ---- Phase QKV: Q, K projections (fp8 DoubleRow) ----
            qkv_ps_cm = tc.tile_pool(name="qkv_ps", bufs=4, space="PSUM")
            qkv_ps = qkv_ps_cm.__enter__()
            for w8s, b_s, dst in ((wq8_s, bq_s, qt8_s), (wk8_s, bk_s, kt8_s)):
                for g in range(2):
                    for kk in range(4):
                        pss = [
                            qkv_ps.tile([P, 512], F32, tag="qkvps", name="ps_qk")
                            for _ in range(4)
                        ]
                        for h in range(2):
                            lhs3 = re2(w8s[h])[:, :, _ts(kk, P)]
                            for gi in range(4):
                                ib = 4 * g + gi
                                nc.tensor.matmul(
                                    pss[gi],
                                    lhsT=lhs3,
                                    rhs=re2(x8_s[h])[:, :, _ts(ib, 512)],
                                    start=(h == 0),
                                    stop=(h == 1),
                                    perf_mode=DR,
                                )
                        for gi in range(4):
                            ib = 4 * g + gi
                            # dst slice: head chunk kk plane, cols ib*512
                            nc.vector.tensor_scalar(
                                dst[kk // 2][
                                    :, (kk % 2) * T + ib * 512 : (kk % 2) * T + ib * 512 + 512
                                ],
                                pss[gi],
                                1.0 / C4,
                                b_s[:, kk : kk + 1],
                                op0=mybir.AluOpType.mult,
                                op1=mybir.AluOpType.add,
                            )
            qkv_ps_cm.__exit__(None, None, None)

            # ---- Phase 1 + 2 interleaved ----
            s_ps = ctx.enter_context(tc.tile_pool(name="s_ps", bufs=4, space="PSUM"))
            act_ps = ctx.enter_context(
                tc.tile_pool(name="act_ps", bufs=1, space="PSUM")
            )
            zp_pool = ctx.enter_context(tc.tile_pool(name="zp", bufs=4))
            vt_pool = ctx.enter_context(tc.tile_pool(name="vt", bufs=4))
            ob_pool = ctx.enter_context(tc.tile_pool(name="ob", bufs=4))

            # work queue of deferred act-block ops (closures), pumped
            # between strip chunks so TensorE never idles on the exp chain
            pending = []

            def pump(n):
                for _ in range(min(n, len(pending))):
                    pending.pop(0)()

            def emit_v_tile(t):
                # V projection tile t (fp8 DoubleRow), borrows s_ps slot
                ps = s_ps.tile([P, 512], F32, tag="sps", name="ps_v")
                for h in range(2):
                    nc.tensor.matmul(
                        ps,
                        lhsT=re2(x8_s[h])[:, :, _ts(t, P)],
                        rhs=re2(wv8_s[h]),
                        start=(h == 0),
                        stop=(h == 1),
                        perf_mode=DR,
                    )
                nc.vector.tensor_add(v16_s[t], ps, bv_s)

            def enqueue_act_block(ib):
                nm = 2 * (ib + 1)  # pairs m contributing to block ib
                pss = [
                    act_ps.tile([P, 512], F32, tag=f"aps{v}", name=f"aps{v}")
                    for v in range(4)
                ]

                def mk_mm(m, vc):
                    def go():
                        off = 512 * ib - 512 * (m // 2)
                        nc.tensor.matmul(
                            pss[vc],
                            lhsT=re2(v8_s[m])[:, :, _ts(vc, P)],
                            rhs=re2(pt8_s[m])[:, :, off : off + 512],
                            start=(m == 0),
                            stop=(m == nm - 1),
                            perf_mode=DR,
                        )

                    return go

                def mk_ev(vc):
                    def go():
                        # GpSimd can't read PSUM; split eviction Scalar/DVE
                        ob = ob_pool.tile([P, 512], F32, tag="ob", name="ob")
                        if vc % 2 == 0:
                            nc.scalar.copy(ob, pss[vc])
                        else:
                            nc.vector.tensor_copy(ob, pss[vc])
                        nc.sync.dma_start(
                            out=out_d[C + vc * P : C + (vc + 1) * P, _ts(ib, 512)],
                            in_=ob,
                        )

                    return go

                for m in range(nm):
                    for vc in range(4):
                        pending.append(mk_mm(m, vc))
                for vc in range(4):
                    pending.append(mk_ev(vc))

            for t in range(4):
                emit_v_tile(t)

            for jc in range(NTC):
                i0 = P * jc
                a0 = 512 * (jc // 4)
                m, u = jc // 2, jc % 2
                Lm = T - a0
                if jc + 4 < NTC:
                    emit_v_tile(jc + 4)
                r = jc % 4
                if r > 0:
                    # zero the never-written corner [a0, i0)
                    nc.vector.memset(
                        pt8_s[m][:, u * Lm : u * Lm + P * r], 0.0
                    )
                starts = [i0] + list(range(a0 + 512, T, 512))
                nch = len(starts)
                zp = zp_pool.tile([P, NIB], F32, tag="zp", name="zp")
                for g0 in range(0, nch, 4):
                    grp = starts[g0 : g0 + 4]
                    pss = [
                        s_ps.tile([P, 512], F32, tag="sps", name="ps_s")
                        for _ in grp
                    ]
                    for h in range(2):
                        lhs3 = re2(kt8_s[h])[:, :, i0 : i0 + P]
                        for gi, a in enumerate(grp):
                            b = a0 + 512 * (g0 + gi + 1)
                            nc.tensor.matmul(
                                pss[gi][:, 0 : b - a],
                                lhsT=lhs3,
                                rhs=re2(qt8_s[h])[:, :, a:b],
                                start=(h == 0),
                                stop=(h == 1),
                                perf_mode=DR,
                            )
                    for gi, a in enumerate(grp):
                        ci = g0 + gi
                        b = a0 + 512 * (ci + 1)
                        w = b - a
                        if ci == 0:
                            nc.vector.tensor_add(
                                pss[gi][:, 0:P], pss[gi][:, 0:P], mask_s
                            )
                        nc.scalar.activation(
                            pt8_s[m][:, u * Lm + (a - a0) : u * Lm + (b - a0)],
                            pss[gi][:, 0:w],
                            mybir.ActivationFunctionType.Exp,
                            bias=expshift_s[:, 0:1],
                            scale=1.0,
                            accum_out=zp[:, ci : ci + 1],
                        )
                        pump(2)
                z = zp_pool.tile([P, 1], F32, tag="zf", name="z")
                nc.vector.reduce_sum(z, zp[:, 0:nch], axis=mybir.AxisListType.X)
                nc.vector.reciprocal(zr_s[:, jc : jc + 1], z)
                # fold 1/Z_j into V rows; clip +-240 (fp8e4 overflows to
                # inf/NaN, no saturation) then cast fp8 into pair plane
                vt = vt_pool.tile([P, VDIM], F16, tag="vt", name="vt")
                nc.vector.tensor_scalar(
                    vt,
                    v16_s[jc],
                    zr_s[:, jc : jc + 1],
                    FP8MAX,
                    op0=mybir.AluOpType.mult,
                    op1=mybir.AluOpType.min,
                )
                nc.vector.tensor_scalar_max(
                    v8_s[m][:, _ts(u, VDIM)], vt, -FP8MAX
                )
                if jc % 4 == 3:
                    enqueue_act_block(jc // 4)

            while pending:
                pump(len(pending))

    nc.compile()
    return nc


def _host_inputs(x, Wq, bq, Wk, bk, Wv, bv):
    import ml_dtypes

    def f8(a):
        return np.clip(a, -FP8MAX, FP8MAX).astype(ml_dtypes.float8_e4m3)

    wq8 = f8(np.ascontiguousarray(Wq.T))
    wk8 = f8(np.ascontiguousarray(Wk.T))
    wv8 = f8(np.ascontiguousarray(Wv.T))
    bq_h = np.ascontiguousarray((bq / C4).reshape(4, P).T).astype(np.float32)
    bk_h = np.ascontiguousarray((bk / C4).reshape(4, P).T).astype(np.float32)
    bv_h = np.ascontiguousarray(np.tile(bv.astype(np.float32), (P, 1)))
    r = np.arange(P)
    mask = np.where(r[None, :] >= r[:, None], 0.0, MASK_NEG).astype(np.float32)
    in_maps = []
    for b in range(x.shape[0]):
        xb = np.ascontiguousarray(x[b]).astype(np.float32)
        in_maps.append(
            {
                "x8": f8(xb),
                "x32": xb,
                "wq8": wq8,
                "wk8": wk8,
                "wv8": wv8,
                "bq": bq_h,
                "bk": bk_h,
                "bv": bv_h,
                "mask": mask,
            }
        )
    return in_maps


def kernel(x, Wq, bq, Wk, bk, Wv, bv, _trace=False, _tmpdir=None):
    import time as _time

    x = np.asarray(x, dtype=np.float32)
    if "nc" not in _CACHE:
        t0 = _time.time()
        _CACHE["nc"] = build_nc()
        print(f"[kernel] build_nc done in {_time.time() - t0:.1f}s", flush=True)
    nc = _CACHE["nc"]
    in_maps = _host_inputs(
        x,
        np.asarray(Wq, np.float32),
        np.asarray(bq, np.float32),
        np.asarray(Wk, np.float32),
        np.asarray(bk, np.float32),
        np.asarray(Wv, np.float32),
        np.asarray(bv, np.float32),
    )
    t0 = _time.time()
    res = run_bass_kernel_spmd(
        nc, in_maps, core_ids=list(range(8)), trace=_trace, tmpdir=_tmpdir
    )
    print(f"[kernel] run done in {_time.time() - t0:.1f}s", flush=True)
    _CACHE["last_result"] = res
    out = np.stack([r["out"] for r in res.results]).astype(np.float32)
    return out
